# revision 10
# baseline (speedup 1.0000x reference)
"""Distributed Trainium2 kernel for nn_AlgebraicLinear (8, 4096, 256) x (256, 256) linear.

out[b, s, o] = sum_i x[b, s, i] * weight[o, i] + bias[o]

Sharding: pure data-parallel — batch dim (8) maps 1:1 onto the 8 NeuronCores.
Per core the GEMM is M=4096 tokens, K=256, N=256.

Layout: the host passes x[c].T (256, 4096) so the contraction axis i lands on
SBUF partitions with contiguous DMAs (no on-chip transpose). The device
computes out.T tiles (psum [o:128, s:512]) with float32r (FP22) matmuls; bias
is added during the PSUM->SBUF eviction (split across VectorE and ScalarE; it
is a per-partition scalar in this orientation). The host transposes the
returned out.T back. The weight W.T and bias are packed into one (128, 514)
host array so a single DMA loads all constants.

Raw bacc (no TileContext): hand-placed semaphores avoid Tile's multi-usec
end-of-kernel semaphore-reset butterfly. Engine plan:
  Sync   ring: 5 input dma_starts (wb, x0..x3), final out_sem drain wait
  Tensor     : 32 matmuls (16 psum groups of K=2), 8 PSUM banks round-robin
  Vector     : evicts sh=0 half of each output block (tensor_scalar_add bias)
  Scalar     : evicts sh=1 half + issues the block's out-DMA on its own
               HWDGE ring (8 out dma_starts)
"""

import numpy as np

B, S, I, O = 8, 4096, 256, 256
P = 128
SBLK = 1024
NS = S // SBLK    # 4 x-blocks
NH = SBLK // 512  # 2 psum halves per block
KT = I // P       # 2
OT = O // P       # 2
NB = NS * OT      # 8 output blocks
NG = NB * NH      # 16 psum groups
WB_COLS = KT * O + OT  # 514: [k*256+o] weights, then 2 bias cols
N_CORES = 8

_CACHE = {}


def _build():
    if "nc" in _CACHE:
        return _CACHE["nc"]

    import concourse.bass as bass  # noqa: F401
    import concourse.mybir as mybir
    from concourse import bacc
    from contextlib import ExitStack

    f32 = mybir.dt.float32
    f32r = mybir.dt.float32r
    Act = mybir.ActivationFunctionType

    nc = bacc.Bacc("TRN2", target_bir_lowering=False, debug=False,
                   num_devices=N_CORES)

    xT_ext = nc.dram_tensor("xT", [I, S], f32r, kind="ExternalInput")
    wb_ext = nc.dram_tensor("wb", [P, WB_COLS], f32r, kind="ExternalInput")
    out_ext = nc.dram_tensor("out", [O, S], f32, kind="ExternalOutput")

    xT_d = xT_ext.ap().rearrange("(k p) s -> p k s", p=P)      # [128, 2, 4096]
    out_d = out_ext.ap().rearrange("(t p) s -> t p s", p=P)    # [2, 128, 4096]

    with ExitStack() as ctx:
        wb_sb = ctx.enter_context(nc.sbuf_tensor("wb_sb", [P, WB_COLS], f32r))
        x_sb = [ctx.enter_context(nc.sbuf_tensor(f"x_sb{i}", [P, KT, SBLK], f32r))
                for i in range(NS)]
        o_sb = [ctx.enter_context(nc.sbuf_tensor(f"o_sb{i}", [P, SBLK], f32))
                for i in range(NB)]
        scratch = ctx.enter_context(nc.sbuf_tensor("scratch", [P, 1], f32))
        ps = [ctx.enter_context(nc.psum_tensor(f"ps{i}", [P, 512], f32))
              for i in range(8)]

        wb_sem = ctx.enter_context(nc.semaphore("wb_sem"))
        x_sem = [ctx.enter_context(nc.semaphore(f"x_sem{i}"))
                 for i in range(NS)]
        mm_sem = ctx.enter_context(nc.semaphore("mm_sem"))
        dve_sem = ctx.enter_context(nc.semaphore("dve_sem"))
        act_sem = ctx.enter_context(nc.semaphore("act_sem"))
        out_sem = [ctx.enter_context(nc.semaphore(f"out_sem{i}"))
                   for i in range(NB)]

        block = ctx.enter_context(nc.Block())

        def w_ap(k, ot):
            return wb_sb[:, k * O + ot * P:k * O + (ot + 1) * P]

        def bias_ap(ot):
            return wb_sb[:, KT * O + ot:KT * O + ot + 1].bitcast(f32)

        @block.sync
        def _(sp):
            sp.dma_start(out=wb_sb[:], in_=wb_ext.ap()).then_inc(wb_sem, 16)
            for sb in range(NS):
                s0 = sb * SBLK
                sp.dma_start(
                    out=x_sb[sb][:], in_=xT_d[:, :, s0:s0 + SBLK]
                ).then_inc(x_sem[sb], 16)
            # Kernel completion: every output byte landed in DRAM.
            for ob in range(NB):
                sp.wait_ge(out_sem[ob], 16)

        @block.tensor
        def _(pe):
            for g in range(NG):
                sb, ot, sh = g // 4, (g // 2) % 2, g % 2
                if g == 0:
                    pe.wait_ge(wb_sem, 16)
                if ot == 0 and sh == 0:
                    pe.wait_ge(x_sem[sb], 16)
                if g >= 8:
                    ob_prior = (g - 8) // 2
                    if (g - 8) % 2 == 0:
                        pe.wait_ge(dve_sem, ob_prior + 1)
                    else:
                        pe.wait_ge(act_sem, ob_prior + 1)
                bank = ps[g % 8]
                for k in range(KT):
                    mm = nc.tensor.matmul(
                        bank[:],
                        lhsT=w_ap(k, ot),
                        rhs=x_sb[sb][:, k, sh * 512:(sh + 1) * 512],
                        start=(k == 0),
                        stop=(k == KT - 1),
                    )
                mm.then_inc(mm_sem)

        @block.vector
        def _(dve):
            dve.wait_ge(wb_sem, 16)
            for ob in range(NB):
                g = 2 * ob
                ot = ob % 2
                dve.wait_ge(mm_sem, g + 1)
                nc.vector.tensor_scalar_add(
                    o_sb[ob][:, 0:512], ps[g % 8][:], bias_ap(ot)
                ).then_inc(dve_sem)

        @block.scalar
        def _(act):
            # Pre-warm the ACT LUT during the DMA fill phase (scale=0 so the
            # junk result it writes into o_sb[0] is data-independent; the real
            # eviction overwrites it later on this same engine).
            act.wait_ge(wb_sem, 16)
            nc.scalar.activation(scratch[:], wb_sb[:, 0:1].bitcast(f32),
                                 Act.Identity, bias=0.0, scale=0.0)
            for ob in range(NB):
                g = 2 * ob + 1
                sb, ot = ob // 2, ob % 2
                act.wait_ge(mm_sem, g + 1)
                nc.scalar.activation(
                    o_sb[ob][:, 512:1024], ps[g % 8][:], Act.Identity,
                    bias=bias_ap(ot),
                ).then_inc(act_sem)
                act.wait_ge(dve_sem, ob + 1)
                act.wait_ge(act_sem, ob + 1)
                act.dma_start(
                    out=out_d[ot][:, sb * SBLK:(sb + 1) * SBLK],
                    in_=o_sb[ob][:],
                ).then_inc(out_sem[ob], 16)

    nc.compile()
    _CACHE["nc"] = nc
    return nc


def _run(in_maps, trace=False, trace_kwargs=None):
    from concourse.bass_utils import run_bass_kernel_spmd

    nc = _build()
    return run_bass_kernel_spmd(
        nc, in_maps, core_ids=list(range(N_CORES)),
        trace=trace, **(trace_kwargs or {}),
    )


def _make_in_maps(x, weight, bias):
    x = np.asarray(x, dtype=np.float32)
    weight = np.asarray(weight, dtype=np.float32)
    bias = np.asarray(bias, dtype=np.float32)
    # wb[p, k*256+o] = W.T[k*128+p, o] = W[o, k*128+p]; wb[p, 512+t] = bias[t*128+p]
    wb = np.empty((P, WB_COLS), dtype=np.float32)
    wT = weight.T  # (I, O)
    for k in range(KT):
        wb[:, k * O:(k + 1) * O] = wT[k * P:(k + 1) * P, :]
    wb[:, KT * O:] = bias.reshape(OT, P).T
    wb = np.ascontiguousarray(wb)
    in_maps = []
    for c in range(N_CORES):
        in_maps.append({
            "xT": np.ascontiguousarray(x[c].T),
            "wb": wb,
        })
    return in_maps


def kernel(x, weight, bias):
    in_maps = _make_in_maps(x, weight, bias)
    res = _run(in_maps)
    out = np.empty((B, S, O), dtype=np.float32)
    for c in range(N_CORES):
        out[c] = res.results[c]["out"].T
    return out


# revision 11
# speedup vs baseline: 1.1458x; 1.1458x over previous
"""Distributed Trainium2 kernel for nn_AlgebraicLinear (8, 4096, 256) x (256, 256) linear.

out[b, s, o] = sum_i x[b, s, i] * weight[o, i] + bias[o]

Sharding: pure data-parallel — batch dim (8) maps 1:1 onto the 8 NeuronCores.
Per core the GEMM is M=4096 tokens, K=256, N=256.

Layout: the host passes x[c].T (256, 4096) so the contraction axis i lands on
SBUF partitions with contiguous DMAs (no on-chip transpose). The device
computes out.T tiles (psum [o:128, s:512]) with float32r (FP22) matmuls; bias
is added during the PSUM->SBUF eviction (split across VectorE and ScalarE; it
is a per-partition scalar in this orientation). The host transposes the
returned out.T back. The weight W.T and bias are packed into one (128, 514)
host array so a single DMA loads all constants.

Raw bacc (no TileContext): hand-placed semaphores avoid Tile's multi-usec
end-of-kernel semaphore-reset butterfly. Engine plan:
  Sync   ring: 5 input dma_starts (wb, x0..x3), final out_sem drain wait
  Tensor     : 32 matmuls (16 psum groups of K=2), 8 PSUM banks round-robin
  Vector     : evicts sh=0 half of each output block (tensor_scalar_add bias)
  Scalar     : evicts sh=1 half + issues the block's out-DMA on its own
               HWDGE ring (8 out dma_starts)
"""

import numpy as np

B, S, I, O = 8, 4096, 256, 256
P = 128
SBLK = 1024
NS = S // SBLK    # 4 x-blocks
NH = SBLK // 512  # 2 psum halves per block
KT = I // P       # 2
OT = O // P       # 2
NB = NS * OT      # 8 output blocks
NG = NB * NH      # 16 psum groups
WB_COLS = KT * O + OT  # 514: [k*256+o] weights, then 2 bias cols
N_CORES = 8

_CACHE = {}


def _build():
    if "nc" in _CACHE:
        return _CACHE["nc"]

    import concourse.bass as bass  # noqa: F401
    import concourse.mybir as mybir
    from concourse import bacc
    from contextlib import ExitStack

    f32 = mybir.dt.float32
    f32r = mybir.dt.float32r
    Act = mybir.ActivationFunctionType

    nc = bacc.Bacc("TRN2", target_bir_lowering=False, debug=False,
                   num_devices=N_CORES)

    xT_ext = nc.dram_tensor("xT", [I, S], f32r, kind="ExternalInput")
    wb_ext = nc.dram_tensor("wb", [P, WB_COLS], f32r, kind="ExternalInput")
    out_ext = nc.dram_tensor("out", [O, S], f32, kind="ExternalOutput")

    xT_d = xT_ext.ap().rearrange("(k p) s -> p k s", p=P)      # [128, 2, 4096]
    out_d = out_ext.ap().rearrange("(t p) s -> t p s", p=P)    # [2, 128, 4096]

    with ExitStack() as ctx:
        wb_sb = ctx.enter_context(nc.sbuf_tensor("wb_sb", [P, WB_COLS], f32r))
        # x chunks: finer leading chunks so the first matmuls start sooner.
        CH = [512, 512, 1024, 1024, 1024]
        CH_OFF = [0, 512, 1024, 2048, 3072]
        # col-segment (512-wide) index -> (chunk idx, col offset within chunk)
        SEG_CHUNK = [0, 1, 2, 2, 3, 3, 4, 4]
        SEG_OFF = [0, 0, 0, 512, 0, 512, 0, 512]
        x_sb = [ctx.enter_context(nc.sbuf_tensor(f"x_sb{i}", [P, KT, CH[i]], f32r))
                for i in range(len(CH))]
        o_sb = [ctx.enter_context(nc.sbuf_tensor(f"o_sb{i}", [P, SBLK], f32))
                for i in range(NB)]
        scratch = ctx.enter_context(nc.sbuf_tensor("scratch", [P, 1], f32))
        ps = [ctx.enter_context(nc.psum_tensor(f"ps{i}", [P, 512], f32))
              for i in range(8)]

        wb_sem = ctx.enter_context(nc.semaphore("wb_sem"))
        x_sem = [ctx.enter_context(nc.semaphore(f"x_sem{i}"))
                 for i in range(len(CH))]
        mm_sem = ctx.enter_context(nc.semaphore("mm_sem"))
        dve_sem = ctx.enter_context(nc.semaphore("dve_sem"))
        act_sem = ctx.enter_context(nc.semaphore("act_sem"))
        out_sem = [ctx.enter_context(nc.semaphore(f"out_sem{i}"))
                   for i in range(NB)]

        block = ctx.enter_context(nc.Block(no_gpsimd_drain=True))

        def w_ap(k, ot):
            return wb_sb[:, k * O + ot * P:k * O + (ot + 1) * P]

        def bias_ap(ot):
            return wb_sb[:, KT * O + ot:KT * O + ot + 1].bitcast(f32)

        @block.sync
        def _(sp):
            sp.dma_start(out=wb_sb[:], in_=wb_ext.ap()).then_inc(wb_sem, 16)
            for c in range(len(CH)):
                s0 = CH_OFF[c]
                sp.dma_start(
                    out=x_sb[c][:], in_=xT_d[:, :, s0:s0 + CH[c]]
                ).then_inc(x_sem[c], 16)
            # Kernel completion: every output byte landed in DRAM.
            for ob in range(NB):
                sp.wait_ge(out_sem[ob], 16)

        @block.tensor
        def _(pe):
            waited_chunks = set()
            for g in range(NG):
                sb, ot, sh = g // 4, (g // 2) % 2, g % 2
                seg = sb * 2 + sh
                c, coff = SEG_CHUNK[seg], SEG_OFF[seg]
                if g == 0:
                    pe.wait_ge(wb_sem, 16)
                if c not in waited_chunks:
                    waited_chunks.add(c)
                    pe.wait_ge(x_sem[c], 16)
                if g >= 8:
                    ob_prior = (g - 8) // 2
                    if (g - 8) % 2 == 0:
                        pe.wait_ge(dve_sem, ob_prior + 1)
                    else:
                        pe.wait_ge(act_sem, ob_prior + 1)
                bank = ps[g % 8]
                for k in range(KT):
                    mm = nc.tensor.matmul(
                        bank[:],
                        lhsT=w_ap(k, ot),
                        rhs=x_sb[c][:, k, coff:coff + 512],
                        start=(k == 0),
                        stop=(k == KT - 1),
                    )
                mm.then_inc(mm_sem)

        @block.vector
        def _(dve):
            dve.wait_ge(wb_sem, 16)
            for ob in range(NB):
                g = 2 * ob
                ot = ob % 2
                dve.wait_ge(mm_sem, g + 1)
                nc.vector.tensor_scalar_add(
                    o_sb[ob][:, 0:512], ps[g % 8][:], bias_ap(ot)
                ).then_inc(dve_sem)

        @block.scalar
        def _(act):
            # Pre-warm the ACT LUT during the DMA fill phase (scale=0 so the
            # junk result it writes into o_sb[0] is data-independent; the real
            # eviction overwrites it later on this same engine).
            act.wait_ge(wb_sem, 16)
            nc.scalar.activation(scratch[:], wb_sb[:, 0:1].bitcast(f32),
                                 Act.Identity, bias=bias_ap(0), scale=0.0)
            for ob in range(NB):
                g = 2 * ob + 1
                sb, ot = ob // 2, ob % 2
                act.wait_ge(mm_sem, g + 1)
                nc.scalar.activation(
                    o_sb[ob][:, 512:1024], ps[g % 8][:], Act.Identity,
                    bias=bias_ap(ot),
                ).then_inc(act_sem)
                act.wait_ge(dve_sem, ob + 1)
                act.wait_ge(act_sem, ob + 1)
                act.dma_start(
                    out=out_d[ot][:, sb * SBLK:(sb + 1) * SBLK],
                    in_=o_sb[ob][:],
                ).then_inc(out_sem[ob], 16)

    nc.compile()
    _CACHE["nc"] = nc
    return nc


def _run(in_maps, trace=False, trace_kwargs=None):
    from concourse.bass_utils import run_bass_kernel_spmd

    nc = _build()
    return run_bass_kernel_spmd(
        nc, in_maps, core_ids=list(range(N_CORES)),
        trace=trace, **(trace_kwargs or {}),
    )


def _make_in_maps(x, weight, bias):
    x = np.asarray(x, dtype=np.float32)
    weight = np.asarray(weight, dtype=np.float32)
    bias = np.asarray(bias, dtype=np.float32)
    # wb[p, k*256+o] = W.T[k*128+p, o] = W[o, k*128+p]; wb[p, 512+t] = bias[t*128+p]
    wb = np.empty((P, WB_COLS), dtype=np.float32)
    wT = weight.T  # (I, O)
    for k in range(KT):
        wb[:, k * O:(k + 1) * O] = wT[k * P:(k + 1) * P, :]
    wb[:, KT * O:] = bias.reshape(OT, P).T
    wb = np.ascontiguousarray(wb)
    in_maps = []
    for c in range(N_CORES):
        in_maps.append({
            "xT": np.ascontiguousarray(x[c].T),
            "wb": wb,
        })
    return in_maps


def kernel(x, weight, bias):
    in_maps = _make_in_maps(x, weight, bias)
    res = _run(in_maps)
    out = np.empty((B, S, O), dtype=np.float32)
    for c in range(N_CORES):
        out[c] = res.results[c]["out"].T
    return out
